# revision 1
# baseline (speedup 1.0000x reference)
"""2-layer GAT (PyG-style, eval mode) on 8 Trainium2 NeuronCores via Bass/Tile.

Architecture (dst-sharded, destination-bucketed, dma_gather based):
  - Destination nodes are sharded across 8 cores (12500 each, padded 12544).
  - Per core, dst nodes are sorted by in-degree and grouped into 98 blocks of
    128 (the partition dim). Each node's incoming edges occupy "slots" along
    the free dim; per (block, src-range group) the slot count is the block max
    degree (degree sorting keeps the max near the mean).
  - Per-edge source features are fetched with InstDMAGatherAnt (the custom
    GPSIMD gather): 256B fp32 rows [a_src | xp], int16 indices => 4 source
    groups of 25088 table rows; gathers round-robin over 4 SWDGE queues.
  - Segment softmax folds into: p = exp(leaky_relu(a_s + a_d)); S = sum_j p;
    out = (sum_j p * xp) / S  (no segment-max: attention logits are O(1), so
    exp cannot overflow; this matches the reference up to fp rounding).
  - Padded slots point at a "dummy" table row whose a_s = -1e30 => p = 0.
  - Node-feature tables are exchanged compactly with AllGather and expanded
    to the 256B-aligned gather layout on device with fat DMAs + DVE copies.
"""

import sys

sys.path.insert(0, "/opt/trn_rl_repo")

import numpy as np

import concourse.bass as bass
import concourse.bacc as bacc
import concourse.mybir as mybir
from concourse.tile import TileContext
from concourse import library_config
from concourse.bass_utils import run_bass_kernel_spmd

F32 = mybir.dt.float32
I16 = mybir.dt.int16

AX = mybir.AxisListType.X
ALU = mybir.AluOpType
ACTF = mybir.ActivationFunctionType

NEG_BIG = -1.0e30


class Cfg:
    def __init__(self, N=100000, E=3200000, F_IN=512, ncores=8):
        self.N = N
        self.E = E
        self.F_IN = F_IN
        self.H1, self.C1 = 4, 8
        self.H2, self.C2 = 1, 8
        self.NEG_SLOPE = 0.2
        self.NCORES = ncores
        assert N % ncores == 0
        self.NPC = N // ncores                      # real nodes per core
        self.NPAD = ((self.NPC + 127) // 128) * 128  # padded
        self.NBLK = self.NPAD // 128
        self.GROUPS = 4
        assert (self.NPAD * ncores) % self.GROUPS == 0
        self.GROUP_ROWS = self.NPAD * ncores // self.GROUPS  # table rows/group
        assert self.GROUP_ROWS <= 32767
        # compact row layouts (fp32 words)
        self.ROW1C = 4 + self.H1 * self.C1   # 36: [a_s1(4) | xp1(32)]
        self.ROW2C = 1 + self.C2             # 9:  [a_s2(1) | xp2(8)]
        self.ROWP = 64                       # padded row: 256B


# ----------------------------------------------------------------------------
# Host-side preprocessing: sharding, permutation, slot/idx construction
# ----------------------------------------------------------------------------
class Plan:
    """Host plan: per-core node permutation + slot structure (shared by both
    layers) + gather index planes."""

    def __init__(self, cfg: Cfg, edge_index: np.ndarray):
        c = cfg
        src0 = edge_index[0].astype(np.int64)
        dst0 = edge_index[1].astype(np.int64)
        loop = np.arange(c.N, dtype=np.int64)
        src = np.concatenate([src0, loop])
        dst = np.concatenate([dst0, loop])

        core_of = dst // c.NPC                     # dst shard
        # per-core local structures
        self.perm = []          # perm[c][r] = original local node at rank r
        self.D = np.zeros((c.NBLK, c.GROUPS), dtype=np.int64)  # global maxima
        per_core = []
        for ci in range(c.NCORES):
            m = core_of == ci
            s_c, d_c = src[m], dst[m] - ci * c.NPC
            deg = np.bincount(d_c, minlength=c.NPC)
            order = np.argsort(-deg, kind="stable")   # rank -> orig local
            rank_of = np.empty(c.NPC, dtype=np.int64)
            rank_of[order] = np.arange(c.NPC)
            self.perm.append(order)
            r_c = rank_of[d_c]                        # dst rank
            per_core.append((s_c, r_c))

        # global permuted "rank" of any node (as a source):
        # node n (core k, local l) -> rank_of_k[l]; global row index in the
        # compact/padded table = 12544*k + rc where rc = 98*(rank%128)+rank//128
        self.rank_of_global = np.empty(c.N, dtype=np.int64)
        for ci in range(c.NCORES):
            rank_of = np.empty(c.NPC, dtype=np.int64)
            rank_of[self.perm[ci]] = np.arange(c.NPC)
            self.rank_of_global[ci * c.NPC:(ci + 1) * c.NPC] = ci * c.NPAD + (
                c.NBLK * (rank_of % 128) + rank_of // 128)

        # slot construction, per core
        # edge (src s, dst rank r): block b=r//128, part p=r%128,
        # group g = table_row(s) // GROUP_ROWS
        tbl_row = self.rank_of_global[:]  # table row of node as src
        grp_of_row = tbl_row // c.GROUP_ROWS
        self.core_edges = []
        counts = np.zeros((c.NCORES, c.NBLK, c.GROUPS, 128), dtype=np.int64)
        for ci in range(c.NCORES):
            s_c, r_c = per_core[ci]
            b = r_c // 128
            p = r_c % 128
            g = grp_of_row[s_c]
            np.add.at(counts[ci], (b, g, p), 1)
            self.core_edges.append((s_c, r_c, b, p, g))
        self.D = counts.max(axis=(0, 3))  # [NBLK, GROUPS] global max
        self.D = np.maximum(self.D, 1)
        self.slots_bg = 128 * self.D      # slots per (b, g)
        self.tot_slots = int(self.slots_bg.sum())

        # dummy row per group: core (pad row): use padded-rank NPC (first pad)
        # of the first core whose rows fall in group g.
        # global padded row index for (core k, rank q) = k*NPAD + rc(q)
        self.dummy_local = np.zeros(c.GROUPS, dtype=np.int64)
        for g in range(c.GROUPS):
            # find a pad row in group g: core k occupies rows
            # [k*NPAD, (k+1)*NPAD); group g covers [g*GR, (g+1)*GR)
            k = (g * c.GROUP_ROWS) // c.NPAD
            q = c.NPC  # first pad rank (only exists if NPAD > NPC)
            if c.NPAD == c.NPC:
                q = c.NPC - 1  # fallback: last rank (tiny test configs)
            rc = c.NBLK * (q % 128) + q // 128
            row = k * c.NPAD + rc
            assert g * c.GROUP_ROWS <= row < (g + 1) * c.GROUP_ROWS
            self.dummy_local[g] = row - g * c.GROUP_ROWS

        # gather idx planes per core: concat over (b, g) of wrapped
        # [128, 8*D] int16 planes
        self.idx_planes = []
        for ci in range(c.NCORES):
            s_c, r_c, b, p, g = self.core_edges[ci]
            plane = np.empty((128, 8 * int(self.D.sum())), dtype=np.int16)
            segs = []
            for bb in range(c.NBLK):
                for gg in range(c.GROUPS):
                    Dn = int(self.D[bb, gg])
                    nslots = 128 * Dn
                    sel = (b == bb) & (g == gg)
                    pp = p[sel]
                    loc = tbl_row[s_c[sel]] - gg * c.GROUP_ROWS
                    # slot index within (bb, gg): node p gets slots p, 128+p,...
                    # order within node arbitrary; use cumcount
                    ordr = np.argsort(pp, kind="stable")
                    pp_s = pp[ordr]
                    loc_s = loc[ordr]
                    # j-th edge of partition pp -> slot j*128+pp
                    jj = np.arange(pp_s.size) - np.searchsorted(pp_s, pp_s)
                    slot = jj * 128 + pp_s
                    arr = np.full(nslots, self.dummy_local[gg], dtype=np.int16)
                    arr[slot] = loc_s.astype(np.int16)
                    segs.append(arr.reshape(-1, 16).T)  # [16, nslots/16]
            wrapped = np.concatenate(segs, axis=1)
            plane = np.tile(wrapped, (8, 1)).astype(np.int16)
            self.idx_planes.append(plane)
        self.idx_cols = self.idx_planes[0].shape[1]


# ----------------------------------------------------------------------------
# Device kernel builder (one program, SPMD on 8 cores)
# ----------------------------------------------------------------------------
def build_kernel(cfg: Cfg, plan: Plan):
    c = cfg
    NB = c.NBLK
    TROWS = c.NPAD * c.NCORES          # padded table rows (100352)
    nc = bacc.Bacc(num_swdge_queues=4, num_devices=c.NCORES)

    # ---- inputs ----
    xT = nc.dram_tensor("xT", [c.F_IN, c.NPAD], F32, kind="ExternalInput")
    w1e = nc.dram_tensor("w1e", [c.F_IN, 40], F32, kind="ExternalInput")
    w2e = nc.dram_tensor("w2e", [32, 12], F32, kind="ExternalInput")
    b1r = nc.dram_tensor("b1r", [128, 32], F32, kind="ExternalInput")
    b2r = nc.dram_tensor("b2r", [128, 8], F32, kind="ExternalInput")
    padneg = nc.dram_tensor("padneg", [128, 1], F32, kind="ExternalInput")
    padone = nc.dram_tensor("padone", [128, 1], F32, kind="ExternalInput")
    idxt = nc.dram_tensor("idxt", [128, plan.idx_cols], I16, kind="ExternalInput")
    y = nc.dram_tensor("y", [128, NB * 8], F32, kind="ExternalOutput")

    # ---- internal DRAM ----
    tc1_in = nc.dram_tensor("tc1_in", [128 * NB * c.ROW1C], F32, kind="Internal")
    tc1_full = nc.dram_tensor("tc1_full", [TROWS * c.ROW1C], F32,
                              kind="Internal", addr_space="Shared")
    tbl1 = nc.dram_tensor("tbl1", [TROWS, c.ROWP], F32, kind="Internal")
    tc2_in = nc.dram_tensor("tc2_in", [128 * NB * c.ROW2C], F32, kind="Internal")
    tc2_full = nc.dram_tensor("tc2_full", [TROWS * c.ROW2C], F32,
                              kind="Internal", addr_space="Shared")
    tbl2 = nc.dram_tensor("tbl2", [TROWS, c.ROWP], F32, kind="Internal")

    replica_groups = [list(range(c.NCORES))]

    with TileContext(nc) as tc:
        with (
            tc.tile_pool(name="persist", bufs=1) as pp,
            tc.tile_pool(name="gidx", bufs=8) as gip,
            tc.tile_pool(name="work", bufs=3) as wp,
        ):
            with tc.high_priority():
                nc.gpsimd.load_library(library_config.mlp)

            # persistent SBUF
            a_d1 = pp.tile([128, NB * 4], F32)       # a_d layer1 (node-major)
            a_d2 = pp.tile([128, NB], F32)           # a_d layer2
            comp1 = pp.tile([128, NB * c.ROW1C], F32)  # compact xps1 slice
            outcat = pp.tile([128, NB * 36], F32)    # L1: [S(4) | out_un(32)]
            hcat = pp.tile([128, NB * 32], F32)      # h after elu
            comp2 = pp.tile([128, NB * c.ROW2C], F32)
            out2cat = pp.tile([128, NB * 9], F32)    # L2: [S2(1) | out2_un(8)]
            b1t = pp.tile([128, 32], F32)
            b2t = pp.tile([128, 8], F32)
            pnt = pp.tile([128, 1], F32)
            pot = pp.tile([128, 1], F32)
            nc.sync.dma_start(b1t[:], b1r[:])
            nc.sync.dma_start(b2t[:], b2r[:])
            nc.sync.dma_start(pnt[:], padneg[:])
            nc.sync.dma_start(pot[:], padone[:])

            # ---------------- Phase A: xps1 = [x @ W1ext] ----------------
            w1sb = pp.tile([128, 4, 40], F32)
            nc.sync.dma_start(w1sb[:], w1e[:].rearrange("(k p) n -> p k n", p=128))
            ident = pp.tile([128, 128], F32)
            from concourse.masks import make_identity
            make_identity(nc, ident[:])

            NT = 512  # nodes per matmul tile
            mp_cm = tc.tile_pool(name="mm", bufs=3)
            mp = mp_cm.__enter__()
            psp_cm = tc.tile_pool(name="mmpa", bufs=2, space="PSUM"); psp = psp_cm.__enter__()
            for t0 in range(0, c.NPAD, NT):
                nt = min(NT, c.NPAD - t0)
                xtile = mp.tile([128, 4, NT], F32, tag="xt")
                nc.sync.dma_start(xtile[:, :, :nt],
                                  xT[:, t0:t0 + nt].rearrange("(k p) n -> p k n", p=128))
                ps = psp.tile([40, NT], F32, tag="mm1")
                for k in range(4):
                    nc.tensor.matmul(ps[:, :nt], w1sb[:, k, :], xtile[:, k, :nt],
                                     start=(k == 0), stop=(k == 3))
                xpsT = mp.tile([40, NT], F32, tag="xpsT")
                nc.scalar.copy(xpsT[:, :nt], ps[:, :nt])
                # transpose per 128-node chunk -> node-major
                for s0 in range(0, nt, 128):
                    b = (t0 + s0) // 128
                    pst = psp.tile([128, 40], F32, tag="tr1")
                    nc.tensor.transpose(pst[:], xpsT[:, s0:s0 + 128], ident[:40, :40])
                    nm = wp.tile([128, 40], F32, tag="nm")
                    nc.scalar.copy(nm[:], pst[:])
                    if b == NB - 1 and c.NPAD > c.NPC:
                        # pad nodes get a_s = -1e30 (additive mask input)
                        nc.vector.tensor_add(
                            nm[:, 32:36], nm[:, 32:36],
                            pnt[:].broadcast_to([128, 4]))
                    # a_d -> resident
                    nc.vector.tensor_copy(a_d1[:, b * 4:(b + 1) * 4], nm[:, 36:40])
                    # compact row [a_s | xp]
                    nc.vector.tensor_copy(
                        comp1[:, b * c.ROW1C:b * c.ROW1C + 4], nm[:, 32:36])
                    nc.vector.tensor_copy(
                        comp1[:, b * c.ROW1C + 4:(b + 1) * c.ROW1C], nm[:, 0:32])

            psp_cm.__exit__(None, None, None)
            mp_cm.__exit__(None, None, None)
            # write compact slice (partition-major) + allgather + expand
            nc.sync.dma_start(
                tc1_in[:].rearrange("(p w) -> p w", p=128), comp1[:])
            nc.gpsimd.collective_compute(
                "AllGather", ALU.bypass,
                ins=[tc1_in[:]], outs=[tc1_full[:]],
                replica_groups=replica_groups,
            )
            _expand_table(nc, tc, wp, cfg, tc1_full, tbl1, c.ROW1C)

            # ---------------- L1 edge phase ----------------
            gp_cm = tc.tile_pool(name="gat", bufs=3)
            gp = gp_cm.__enter__()
            tp_cm = tc.tile_pool(name="tmp", bufs=2)
            tpool = tp_cm.__enter__()
            _edge_layer(nc, tc, cfg, plan, gp, gip, wp, tpool, idxt, tbl1,
                        a_d1, outcat, layer=1)

            # ---------------- L1 epilogue: h, xps2 ----------------
            w2sb = pp.tile([32, 12], F32)
            nc.sync.dma_start(w2sb[:], w2e[:])
            psp_cm = tc.tile_pool(name="mmpb", bufs=2, space="PSUM")
            psp = psp_cm.__enter__()
            if c.NPAD > c.NPC:
                sl = outcat[:, (NB - 1) * 36:(NB - 1) * 36 + 4]
                nc.vector.tensor_add(sl, sl, pot[:].broadcast_to([128, 4]))
            for b in range(NB):
                S = outcat[:, b * 36:b * 36 + 4]
                ou = outcat[:, b * 36 + 4:(b + 1) * 36]
                r = wp.tile([128, 4], F32, tag="r1")
                nc.vector.reciprocal(r[:], S)
                z = wp.tile([128, 32], F32, tag="z")
                # z = out_un * r (broadcast r over 8 channels)
                nc.vector.tensor_tensor(
                    out=z[:].rearrange("p (h c) -> p h c", h=4),
                    in0=ou.rearrange("p (h c) -> p h c", h=4),
                    in1=r[:, :, None].broadcast_to([128, 4, 8]),
                    op=ALU.mult)
                nc.vector.tensor_add(z[:], z[:], b1t[:])
                # elu: h = max(z,0) + exp(min(z,0)) - 1
                mneg = wp.tile([128, 32], F32, tag="mneg")
                nc.vector.tensor_scalar(out=mneg[:], in0=z[:], scalar1=0.0,
                                        scalar2=None, op0=ALU.min)
                q = wp.tile([128, 32], F32, tag="q")
                nc.scalar.activation(q[:], mneg[:], ACTF.Exp)
                h = hcat[:, b * 32:(b + 1) * 32]
                nc.vector.tensor_scalar(out=h, in0=z[:], scalar1=0.0,
                                        scalar2=None, op0=ALU.max)
                nc.vector.tensor_add(h, h, q[:])
                nc.vector.tensor_scalar_add(h, h, -1.0)
                # xps2 = h @ W2ext : transpose h -> [32, 128]
                psh = psp.tile([32, 128], F32, tag="trh")
                nc.tensor.transpose(psh[:], h, ident[:])
                hT = wp.tile([32, 128], F32, tag="hT")
                nc.scalar.copy(hT[:], psh[:])
                ps2 = psp.tile([12, 128], F32, tag="mm2")
                nc.tensor.matmul(ps2[:], w2sb[:], hT[:], start=True, stop=True)
                x2T = wp.tile([12, 128], F32, tag="x2T")
                nc.scalar.copy(x2T[:], ps2[:])
                ps3 = psp.tile([128, 12], F32, tag="tr2")
                nc.tensor.transpose(ps3[:], x2T[:], ident[:12, :12])
                nm2 = wp.tile([128, 12], F32, tag="nm2")
                nc.scalar.copy(nm2[:], ps3[:])
                if b == NB - 1 and c.NPAD > c.NPC:
                    nc.vector.tensor_add(nm2[:, 8:9], nm2[:, 8:9], pnt[:])
                nc.vector.tensor_copy(a_d2[:, b:b + 1], nm2[:, 9:10])
                # compact row2 = [a_s2 | xp2]
                nc.vector.tensor_copy(comp2[:, b * 9:b * 9 + 1], nm2[:, 8:9])
                nc.vector.tensor_copy(comp2[:, b * 9 + 1:(b + 1) * 9], nm2[:, 0:8])

            psp_cm.__exit__(None, None, None)
            nc.sync.dma_start(
                tc2_in[:].rearrange("(p w) -> p w", p=128), comp2[:])
            nc.gpsimd.collective_compute(
                "AllGather", ALU.bypass,
                ins=[tc2_in[:]], outs=[tc2_full[:]],
                replica_groups=replica_groups,
            )
            _expand_table(nc, tc, wp, cfg, tc2_full, tbl2, c.ROW2C)

            # ---------------- L2 edge phase ----------------
            _edge_layer(nc, tc, cfg, plan, gp, gip, wp, tpool, idxt, tbl2,
                        a_d2, out2cat, layer=2)
            tp_cm.__exit__(None, None, None)
            gp_cm.__exit__(None, None, None)

            # ---------------- L2 epilogue ----------------
            if c.NPAD > c.NPC:
                sl2 = out2cat[:, (NB - 1) * 9:(NB - 1) * 9 + 1]
                nc.vector.tensor_add(sl2, sl2, pot[:])
            for b in range(NB):
                S2 = out2cat[:, b * 9:b * 9 + 1]
                ou2 = out2cat[:, b * 9 + 1:(b + 1) * 9]
                r2 = wp.tile([128, 1], F32, tag="r2")
                nc.vector.reciprocal(r2[:], S2)
                fo = wp.tile([128, 8], F32, tag="fo")
                nc.vector.tensor_scalar(out=fo[:], in0=ou2, scalar1=r2[:],
                                        scalar2=None, op0=ALU.mult)
                nc.vector.tensor_add(fo[:], fo[:], b2t[:])
                nc.sync.dma_start(y[:, b * 8:(b + 1) * 8], fo[:])

    nc.finalize()
    return nc



def _dma_gather_raw(gps, out_ap, in_ap, idxs_ap, num_idxs, elem_size,
                    elem_step, queue_num):
    """bass.BassGpSimd.dma_gather with the elem_size%256 assert relaxed to %4
    (the Q7 ucode handles arbitrary element lengths; verified on HW)."""
    from concourse import ap_utils
    from concourse.bass import MemorySpace
    import concourse.mybir as mb

    assert idxs_ap.dtype == I16
    assert in_ap.dtype == out_ap.dtype
    elem_size_bytes = elem_size * mb.dt.size(in_ap.dtype)
    assert elem_size_bytes > 0 and elem_size_bytes % 4 == 0
    assert in_ap.space == MemorySpace.DRAM
    assert idxs_ap.space == MemorySpace.SBUF
    assert out_ap.space == MemorySpace.SBUF
    assert ap_utils.ap_is_contiguous(out_ap.ap[1:])
    assert ap_utils.ap_is_contiguous(idxs_ap.ap[1:])
    assert in_ap.ap[-1][1] == out_ap.ap[-1][1] == elem_size
    assert out_ap.ap[0][1] * out_ap.ap[1][1] == ((num_idxs + 127) // 128) * 128
    assert in_ap.ap[0][0] == elem_step
    stride_bytes = elem_step * mb.dt.size(in_ap.dtype)
    assert stride_bytes % 256 == 0
    stride_bytes_256 = stride_bytes // 256
    assert stride_bytes_256 < 256

    _in_ap = gps.lower_ap_dma(in_ap, for_custom_bir_dma=True)
    _idxs_ap = gps.lower_ap(idxs_ap)
    _out_ap = gps.lower_ap(out_ap)
    return gps.add_instruction(
        mb.InstDMAGatherAnt(
            name=gps.bass.get_next_instruction_name(),
            ins=[*_in_ap, _idxs_ap,
                 gps.lower_val_access(gps.to_reg(num_idxs))],
            outs=[_out_ap],
            transpose=False,
            num_idxs=num_idxs,
            elem_size=elem_size,
            stride_bytes_256=stride_bytes_256,
            gen_mode=0,
            single_packet=False,
            queue_num=queue_num,
        )
    )


def _expand_table(nc, tc, wp_unused, cfg, compact_dram, padded_dram, roww):
    """Expand compact rows [TROWS, roww] (flat) to 256B rows [TROWS, 64].
    Partition-flat processing: partition p owns rows [p*R128, (p+1)*R128)."""
    c = cfg
    TROWS = c.NPAD * c.NCORES
    R128 = TROWS // 128            # compact rows per partition
    CH = 16 if (c.NPAD * c.NCORES) % (128 * 16) == 0 else 4  # chunks
    if R128 % CH != 0:
        CH = 4
    assert R128 % CH == 0
    rch = R128 // CH
    ep_cm = tc.tile_pool(name=f"exp{roww}", bufs=1)
    ep = ep_cm.__enter__()
    for ch in range(CH):
        src = compact_dram[:].rearrange("(p r w) -> p r w", p=128, w=roww)
        ct = ep.tile([128, rch, roww], F32, tag="exp_in")
        nc.sync.dma_start(ct[:], src[:, ch * rch:(ch + 1) * rch, :])
        pt = ep.tile([128, rch, c.ROWP], F32, tag="exp_out")
        nc.vector.memset(pt[:, :, roww:], 0.0)
        nc.vector.tensor_copy(
            pt[:, :, :roww], ct[:])
        dstv = padded_dram[:].rearrange("(p r) w -> p r w", p=128)
        nc.sync.dma_start(dstv[:, ch * rch:(ch + 1) * rch, :], pt[:])
    ep_cm.__exit__(None, None, None)


def _edge_layer(nc, tc, cfg, plan, gp, gip, wp, tpool, idxt, tbl, a_d, outcat,
                layer):
    """Edge phase: per (block, group) gather + attention + aggregation."""
    c = cfg
    H = c.H1 if layer == 1 else c.H2        # heads
    CC = c.C1 if layer == 1 else c.C2       # channels/head
    aw = 4 if layer == 1 else 1             # a_s words at row start
    xw = H * CC                             # xp words
    idx_off = 0
    for b in range(c.NBLK):
        Dt = int(plan.D[b].sum())           # total slots/partition this block
        # gather all groups into one G tile
        Dmax = int(plan.D.sum(1).max())
        RW = 4 + xw                        # gathered words per row
        Gf = gp.tile([128, Dmax, RW], F32, tag=f"G{layer}")
        G = Gf[:, :Dt, :]
        # one idx DMA per block (group segments are adjacent in idxt)
        itf = gip.tile([128, 8 * Dmax], I16, tag="it")
        itb = itf[:, :8 * Dt]
        nc.sync.dma_start(itb[:], idxt[:, idx_off:idx_off + 8 * Dt])
        idx_off += 8 * Dt
        off = 0
        for g in range(c.GROUPS):
            Dg = int(plan.D[b, g])
            nsl = 128 * Dg
            _dma_gather_raw(
                nc.gpsimd,
                G[:, off:off + Dg, :],
                tbl[g * c.GROUP_ROWS:(g + 1) * c.GROUP_ROWS, :RW],
                itb[:, 8 * off:8 * (off + Dg)], nsl, RW, c.ROWP,
                queue_num=g % 4,
            )
            off += Dg
        # e = a_s + a_d on ScalarE (fused add via Identity bias; strided read
        # cost lands on the otherwise-idle ACT engine)
        Hm = cfg.H1
        epf = wp.tile([128, Hm, Dmax], F32, tag="e")
        ep = epf[:, :H, :Dt]
        for h in range(H):
            nc.scalar.activation(
                ep[:, h, :], G[:, :, h], ACTF.Identity,
                bias=a_d[:, b * H + h:b * H + h + 1], scale=1.0)
        # p = exp(leaky_relu(e)): lrelu = max(e,0) + slope*min(e,0)
        lrf = wp.tile([128, Hm, Dmax], F32, tag="lr")
        lr = lrf[:, :H, :Dt]
        nc.vector.tensor_scalar(out=lr[:], in0=ep[:], scalar1=0.0,
                                scalar2=c.NEG_SLOPE, op0=ALU.min,
                                op1=ALU.mult)
        pposf = wp.tile([128, Hm, Dmax], F32, tag="ppos")
        ppos = pposf[:, :H, :Dt]
        nc.vector.tensor_scalar(out=ppos[:], in0=ep[:], scalar1=0.0,
                                scalar2=None, op0=ALU.max)
        nc.vector.tensor_add(lr[:], lr[:], ppos[:])
        pvf = wp.tile([128, Hm, Dmax], F32, tag="p")
        pv = pvf[:, :H, :Dt]
        nc.scalar.activation(pv[:], lr[:], ACTF.Exp)
        # S = sum_j p  -> outcat[:, b, 0:H]
        nc.vector.tensor_reduce(
            out=outcat[:, b * (H + xw):b * (H + xw) + H],
            in_=pv[:], op=ALU.add, axis=AX)
        # msg = p (bcast over CC) * xp ; out_un = sum_j msg
        tmpf = tpool.tile([128, c.H1 * c.C1, Dmax], F32, tag="tmp")
        tmp = tmpf[:, :H * CC, :Dt]
        nc.vector.tensor_tensor(
            out=tmp[:].rearrange("p (h c) d -> p h c d", h=H),
            in0=pv[:, :, None, :].broadcast_to([128, H, CC, Dt]),
            in1=G[:, :, aw:aw + xw].rearrange("p d (h c) -> p h c d", h=H),
            op=ALU.mult)
        nc.vector.tensor_reduce(
            out=outcat[:, b * (H + xw) + H:(b + 1) * (H + xw)],
            in_=tmp[:], op=ALU.add, axis=AX)


# ----------------------------------------------------------------------------
# Host wrapper
# ----------------------------------------------------------------------------
def _build_w1ext(W1, att_src1, att_dst1):
    # [W1 | W1@As | W1@Ad]: As[j, h] = att_src1[h, j%C] if j//C==h
    H, C = att_src1.shape
    As = np.zeros((H * C, H), np.float32)
    Ad = np.zeros((H * C, H), np.float32)
    for h in range(H):
        As[h * C:(h + 1) * C, h] = att_src1[h]
        Ad[h * C:(h + 1) * C, h] = att_dst1[h]
    return np.concatenate([W1, W1 @ As, W1 @ Ad], axis=1).astype(np.float32)


def _build_w2ext(W2, att_src2, att_dst2):
    H, C = att_src2.shape
    As = att_src2.reshape(C, 1).astype(np.float32)
    Ad = att_dst2.reshape(C, 1).astype(np.float32)
    out = np.concatenate([W2, W2 @ As, W2 @ Ad, np.zeros((32, 2), np.float32)],
                         axis=1)
    return out.astype(np.float32)


def _padneg(cfg):
    v = np.zeros((128, 1), np.float32)
    if cfg.NPAD > cfg.NPC:
        v[cfg.NPC % 128:] = NEG_BIG
    return v


def _padone(cfg):
    v = np.zeros((128, 1), np.float32)
    if cfg.NPAD > cfg.NPC:
        v[cfg.NPC % 128:] = 1.0
    return v


_CACHE = {}
LAST_EXEC_NS = None


def kernel(x, edge_index, W1, att_src1, att_dst1, b1, W2, att_src2, att_dst2,
           b2):
    cfg = Cfg(N=x.shape[0], E=edge_index.shape[1], F_IN=x.shape[1])
    key = ("plan", x.shape, edge_index.shape)
    plan = Plan(cfg, np.asarray(edge_index))
    nc = build_kernel(cfg, plan)

    x = np.asarray(x, dtype=np.float32)
    w1e = _build_w1ext(np.asarray(W1), np.asarray(att_src1), np.asarray(att_dst1))
    w2e = _build_w2ext(np.asarray(W2), np.asarray(att_src2), np.asarray(att_dst2))
    b1r = np.tile(np.asarray(b1, np.float32)[None, :], (128, 1))
    b2r = np.tile(np.asarray(b2, np.float32)[None, :], (128, 1))

    in_maps = []
    for ci in range(cfg.NCORES):
        xs = x[ci * cfg.NPC:(ci + 1) * cfg.NPC][plan.perm[ci]]  # [NPC, F]
        if cfg.NPAD > cfg.NPC:
            xs = np.concatenate(
                [xs, np.zeros((cfg.NPAD - cfg.NPC, cfg.F_IN), np.float32)], 0)
        in_maps.append({
            "xT": np.ascontiguousarray(xs.T),
            "w1e": w1e, "w2e": w2e, "b1r": b1r, "b2r": b2r,
            "padneg": _padneg(cfg), "padone": _padone(cfg),
            "idxt": plan.idx_planes[ci],
        })

    global LAST_EXEC_NS
    want_trace = False
    try:
        from antenv.axon_hooks import get_axon_ntff_profile_hook
        want_trace = get_axon_ntff_profile_hook() is not None
    except ImportError:
        pass
    res = run_bass_kernel_spmd(nc, in_maps, core_ids=list(range(cfg.NCORES)),
                               trace=want_trace)
    LAST_EXEC_NS = res.exec_time_ns

    out = np.empty((cfg.N, 8), np.float32)
    for ci in range(cfg.NCORES):
        yv = res.results[ci]["y"].reshape(128, cfg.NBLK, 8)
        ranks = np.arange(cfg.NPC)
        vals = yv[ranks % 128, ranks // 128, :]        # rank-major
        out[ci * cfg.NPC + plan.perm[ci]] = vals
    return out


if __name__ == "__main__":
    pass



# revision 9
# speedup vs baseline: 1.9299x; 1.9299x over previous
"""2-layer GAT (PyG-style, eval mode) on 8 Trainium2 NeuronCores via Bass/Tile.

v2 architecture (dst-sharded, class-balanced gathers, bf16 tables):
  - Destination nodes sharded across 8 cores (12500 each, padded 12544).
  - Per core, dst nodes are sorted by in-degree and placed into 98 blocks of
    128 (partition dim). Table row of the node at (block b, partition p) is
    rc = 98*p + b (+ core*12544), so its "class" rc%4 == (2p+b)%4 is chosen
    by placement inside a 4-block superblock.
  - The gather for class q addresses rows r===q (mod 4) via base offset
    q*PITCH and stride 4*PITCH (int16 idx = r//4 <= 25087). A greedy
    host-side coloring balances each dst node's in-edge classes, cutting the
    per-(block,class) max-degree slot padding from ~2.06x to ~1.2x.
  - Tables are bf16: L1 rows 64 words (128B pitch): [a_s1(4) | xp1(32)];
    L2 rows 32 words (64B pitch): [a_s2(1) | xp2(8)].
  - Attention on ScalarE: p = Exp(Lrelu(a_s + a_d)) with fused bias + alpha
    and fused accumulation S = sum_j p. DVE only does msg = p*xp and the
    [128, H*C, D] segment reduce.
  - Padded slots point at a per-class "dummy" pad row with a_s = -1e30.
"""

import sys

sys.path.insert(0, "/opt/trn_rl_repo")

import numpy as np

import concourse.bass as bass
import concourse.bacc as bacc
import concourse.mybir as mybir
from concourse.tile import TileContext
from concourse import library_config
from concourse.bass_utils import run_bass_kernel_spmd

F32 = mybir.dt.float32
BF16 = mybir.dt.bfloat16
I16 = mybir.dt.int16

AX = mybir.AxisListType.X
ALU = mybir.AluOpType
ACTF = mybir.ActivationFunctionType

NEG_BIG = -1.0e30
PAD_P0 = 106          # pad slots: blocks NBLK-2, NBLK-1, partitions >= 106


class Cfg:
    def __init__(self, N=100000, E=3200000, F_IN=512, ncores=8):
        self.N = N
        self.E = E
        self.F_IN = F_IN
        self.H1, self.C1 = 4, 8
        self.H2, self.C2 = 1, 8
        self.NEG_SLOPE = 0.2
        self.NCORES = ncores
        assert N % ncores == 0
        self.NPC = N // ncores                       # real nodes per core
        self.NPAD = ((self.NPC + 127) // 128) * 128  # padded (12544)
        self.NBLK = self.NPAD // 128                 # 98
        self.TROWS = self.NPAD * ncores              # 100352
        self.QROWS = self.TROWS // 4                 # 25088 (per class)
        assert self.QROWS <= 32767
        self.PITCH1 = 64    # bf16 words per L1 table row (128B)
        self.PITCH2 = 32    # bf16 words per L2 table row (64B)
        self.RW1 = 36       # gathered words L1: [a_s1(4) | xp1(32)]
        self.RW2 = 10       # gathered words L2: [a_s2(1) | xp2(8) | pad]
        self.ROW1C = 36     # compact words L1
        self.ROW2C = 10     # compact words L2


# ----------------------------------------------------------------------------
# Host-side preprocessing
# ----------------------------------------------------------------------------
class Plan:
    """Sharding, class coloring, placement, slot/idx construction."""

    def __init__(self, cfg: Cfg, edge_index: np.ndarray):
        c = cfg
        # self-loops are handled locally on-device; edges only here
        src = edge_index[0].astype(np.int64)
        dst = edge_index[1].astype(np.int64)
        deg = np.bincount(dst, minlength=c.N)
        NSB = (c.NBLK + 3) // 4                     # superblocks per core

        # ---- superblock of each node (degree-descending order) ----------
        sb_of = np.zeros(c.N, dtype=np.int32)
        core_order = []                             # deg-sorted nodes per core
        for ci in range(c.NCORES):
            nodes = np.arange(ci * c.NPC, (ci + 1) * c.NPC)
            ns = nodes[np.argsort(-deg[nodes], kind="stable")]
            core_order.append(ns)
            sb_of[ns] = ci * NSB + np.minimum(np.arange(c.NPC) // 512,
                                              NSB - 1)

        # per-(core,sb,class) real capacity
        cap = np.full((c.NCORES * NSB, 4), 128, dtype=np.int64)
        # last sb: 2 blocks, 53 real slots per class
        last_real = c.NPC - (NSB - 1) * 512
        cap[NSB - 1::NSB, :] = last_real // 4
        assert last_real % 4 == 0

        # ---- class coloring (greedy + 1 refine pass) --------------------
        o = np.argsort(src, kind="stable")
        dst_sorted = dst[o].astype(np.int32)
        starts = np.searchsorted(src[o], np.arange(c.N + 1))
        cnt = np.zeros((c.N, 4), dtype=np.int16)
        cls = np.full(c.N, -1, dtype=np.int8)
        outdeg = np.bincount(src, minlength=c.N)
        order = np.argsort(-outdeg, kind="stable")
        tgt = np.ceil(deg / 4).astype(np.int16)
        for it in range(2):
            for s in order:
                ds = dst_sorted[starts[s]:starts[s + 1]]
                sb = sb_of[s]
                if it > 0:
                    q0 = cls[s]
                    cnt[ds, q0] -= 1
                    cap[sb, q0] += 1
                cc = cnt[ds, :].astype(np.int32)
                over = (cc + 1 > tgt[ds][:, None]).sum(axis=0) * 1000 \
                    + cc.sum(axis=0)
                for q in np.argsort(over, kind="stable"):
                    if cap[sb, q] > 0:
                        cls[s] = q
                        cap[sb, q] -= 1
                        cnt[ds, q] += 1
                        break
        self.cls = cls

        # ---- placement: node -> (block, partition) ----------------------
        # Within block b: even p -> class b%4, odd p -> class (b+2)%4.
        # Pad slots (no node): blocks NBLK-2/NBLK-1, p >= PAD_P0.
        self.pos_node = []
        b_of = np.empty(c.N, dtype=np.int64)
        p_of = np.empty(c.N, dtype=np.int64)
        for ci in range(c.NCORES):
            ns = core_order[ci]
            posn = np.full((c.NBLK, 128), -1, dtype=np.int64)
            for sbi in range(NSB):
                blocks = range(4 * sbi, min(4 * sbi + 4, c.NBLK))
                lo = sbi * 512
                hi = min(lo + 512, c.NPC)
                sb_nodes = ns[lo:hi]
                slots = {q: [] for q in range(4)}
                for b in blocks:
                    for p in range(128):
                        if b >= c.NBLK - 2 and p >= PAD_P0:
                            continue
                        slots[(2 * p + b) % 4].append((b, p))
                byq = {q: [] for q in range(4)}
                for n in sb_nodes:
                    byq[int(cls[n])].append(n)
                for q in range(4):
                    qn = sorted(byq[q], key=lambda n: -int(cnt[n].max()))
                    assert len(qn) <= len(slots[q])
                    for n, (b, p) in zip(qn, slots[q]):
                        posn[b, p] = n
                        b_of[n] = b
                        p_of[n] = p
            self.pos_node.append(posn)

        # ---- table rows & per-class dummy (pad) rows --------------------
        core_of_node = np.arange(c.N) // c.NPC
        self.row_of = core_of_node * c.NPAD + c.NBLK * p_of + b_of
        self.dummy_idx = np.full(4, -1, dtype=np.int64)
        for b in (c.NBLK - 2, c.NBLK - 1):
            for p in range(PAD_P0, 128):
                q = (2 * p + b) % 4
                if self.dummy_idx[q] < 0:
                    self.dummy_idx[q] = (c.NBLK * p + b) // 4
        assert (self.dummy_idx >= 0).all()

        # ---- per-core counts -> global Dg -------------------------------
        cls_src = cls[src].astype(np.int64)
        idx_src = (self.row_of[src] // 4).astype(np.int64)
        per_core = []
        D_all = np.zeros((c.NCORES, c.NBLK, 4), dtype=np.int64)
        core_of_dst = dst // c.NPC
        for ci in range(c.NCORES):
            m = core_of_dst == ci
            bb = b_of[dst[m]]
            pp = p_of[dst[m]]
            qq = cls_src[m]
            ii = idx_src[m]
            counts = np.zeros((c.NBLK, 4, 128), dtype=np.int64)
            np.add.at(counts, (bb, qq, pp), 1)
            D_all[ci] = counts.max(axis=2)
            per_core.append((bb, pp, qq, ii))
        self.Dg = np.maximum(D_all.max(axis=0), 1)   # [NBLK, 4]
        self.Dmax = int(self.Dg.sum(1).max())
        self.tot_slots = int(128 * self.Dg.sum())

        # ---- idx planes -------------------------------------------------
        self.idx_planes = []
        for ci in range(c.NCORES):
            bb, pp, qq, ii = per_core[ci]
            key = (bb * 4 + qq) * 128 + pp
            ordk = np.argsort(key, kind="stable")
            key_s = key[ordk]
            ii_s = ii[ordk]
            pp_s = pp[ordk]
            bq = key_s // 128
            seg_lo = np.searchsorted(bq, np.arange(c.NBLK * 4))
            seg_hi = np.searchsorted(bq, np.arange(c.NBLK * 4) + 1)
            segs = []
            for b in range(c.NBLK):
                for q in range(4):
                    k = b * 4 + q
                    nslots = 128 * int(self.Dg[b, q])
                    arr = np.full(nslots, self.dummy_idx[q], dtype=np.int16)
                    pseg = pp_s[seg_lo[k]:seg_hi[k]]
                    iseg = ii_s[seg_lo[k]:seg_hi[k]]
                    jj = np.arange(len(pseg)) - np.searchsorted(pseg, pseg)
                    arr[jj * 128 + pseg] = iseg.astype(np.int16)
                    segs.append(arr.reshape(-1, 16).T)
            wrapped = np.concatenate(segs, axis=1)
            self.idx_planes.append(np.tile(wrapped, (8, 1)).astype(np.int16))
        self.idx_cols = self.idx_planes[0].shape[1]


# ----------------------------------------------------------------------------
# Device kernel builder (one program, SPMD on 8 cores)
# ----------------------------------------------------------------------------
def build_kernel(cfg: Cfg, plan: Plan):
    c = cfg
    NB = c.NBLK
    nc = bacc.Bacc(num_swdge_queues=4, num_devices=c.NCORES)

    xT = nc.dram_tensor("xT", [c.F_IN, c.NPAD], BF16, kind="ExternalInput")
    w1e = nc.dram_tensor("w1e", [c.F_IN, 40], BF16, kind="ExternalInput")
    w2e = nc.dram_tensor("w2e", [32, 12], F32, kind="ExternalInput")
    b1r = nc.dram_tensor("b1r", [128, 32], F32, kind="ExternalInput")
    b2r = nc.dram_tensor("b2r", [128, 8], F32, kind="ExternalInput")
    padneg = nc.dram_tensor("padneg", [128, 1], F32, kind="ExternalInput")
    padone = nc.dram_tensor("padone", [128, 1], F32, kind="ExternalInput")
    idxt = nc.dram_tensor("idxt", [128, plan.idx_cols], I16,
                          kind="ExternalInput")
    y = nc.dram_tensor("y", [128, NB * 8], F32, kind="ExternalOutput")

    tc1_in = nc.dram_tensor("tc1_in", [128 * NB * c.ROW1C], BF16,
                            kind="Internal")
    tc1_full = nc.dram_tensor("tc1_full", [c.TROWS * c.ROW1C], BF16,
                              kind="Internal", addr_space="Shared")
    tbl1 = nc.dram_tensor("tbl1", [c.TROWS, c.PITCH1], BF16, kind="Internal")
    tc2_in = nc.dram_tensor("tc2_in", [128 * NB * c.ROW2C], BF16,
                            kind="Internal")
    tc2_full = nc.dram_tensor("tc2_full", [c.TROWS * c.ROW2C], BF16,
                              kind="Internal", addr_space="Shared")
    tbl2 = nc.dram_tensor("tbl2", [c.TROWS, c.PITCH2], BF16, kind="Internal")

    replica_groups = [list(range(c.NCORES))]

    with TileContext(nc) as tc:
        with (
            tc.tile_pool(name="persist", bufs=1) as pp,
            tc.tile_pool(name="gidx", bufs=6) as gip,
            tc.tile_pool(name="work", bufs=3) as wp,
        ):
            with tc.high_priority():
                nc.gpsimd.load_library(library_config.mlp)

            a_d1 = pp.tile([128, NB * 4], F32)
            a_d2 = pp.tile([128, NB], F32)
            comp1 = pp.tile([128, NB * c.ROW1C], BF16)
            outcat = pp.tile([128, NB * 36], F32)   # [S(4) | out_un(32)]
            comp2 = pp.tile([128, NB * c.ROW2C], BF16)
            out2cat = pp.tile([128, NB * 9], F32)   # [S2(1) | out2_un(8)]
            b1t = pp.tile([128, 32], F32)
            b2t = pp.tile([128, 8], F32)
            pnt = pp.tile([128, 1], F32)
            pot = pp.tile([128, 1], F32)
            nc.sync.dma_start(b1t[:], b1r[:])
            nc.sync.dma_start(b2t[:], b2r[:])
            nc.sync.dma_start(pnt[:], padneg[:])
            nc.sync.dma_start(pot[:], padone[:])

            # ---------------- Phase A: xps1 = x @ W1ext ----------------
            w1sb = pp.tile([128, 4, 40], BF16)
            nc.sync.dma_start(w1sb[:],
                              w1e[:].rearrange("(k p) n -> p k n", p=128))
            ident = pp.tile([128, 128], F32)
            from concourse.masks import make_identity
            make_identity(nc, ident[:])

            NT = 512
            mp_cm = tc.tile_pool(name="mm", bufs=3)
            mp = mp_cm.__enter__()
            psp_cm = tc.tile_pool(name="mmpa", bufs=2, space="PSUM")
            psp = psp_cm.__enter__()
            for t0 in range(0, c.NPAD, NT):
                nt = min(NT, c.NPAD - t0)
                xtile = mp.tile([128, 4, NT], BF16, tag="xt")
                nc.sync.dma_start(
                    xtile[:, :, :nt],
                    xT[:, t0:t0 + nt].rearrange("(k p) n -> p k n", p=128))
                ps = psp.tile([40, NT], F32, tag="mm1")
                for k in range(4):
                    nc.tensor.matmul(ps[:, :nt], w1sb[:, k, :],
                                     xtile[:, k, :nt],
                                     start=(k == 0), stop=(k == 3))
                xpsT = mp.tile([40, NT], F32, tag="xpsT")
                nc.scalar.copy(xpsT[:, :nt], ps[:, :nt])
                for s0 in range(0, nt, 128):
                    b = (t0 + s0) // 128
                    pst = psp.tile([128, 40], F32, tag="tr1")
                    nc.tensor.transpose(pst[:], xpsT[:, s0:s0 + 128],
                                        ident[:40, :40])
                    nm = wp.tile([128, 40], F32, tag="nm")
                    nc.scalar.copy(nm[:], pst[:])
                    if b >= NB - 2:
                        nc.vector.tensor_add(
                            nm[:, 32:36], nm[:, 32:36],
                            pnt[:].broadcast_to([128, 4]))
                    nc.vector.tensor_copy(a_d1[:, b * 4:(b + 1) * 4],
                                          nm[:, 36:40])
                    nc.vector.tensor_copy(
                        comp1[:, b * c.ROW1C:b * c.ROW1C + 4], nm[:, 32:36])
                    nc.vector.tensor_copy(
                        comp1[:, b * c.ROW1C + 4:(b + 1) * c.ROW1C],
                        nm[:, 0:32])
            psp_cm.__exit__(None, None, None)
            mp_cm.__exit__(None, None, None)

            nc.sync.dma_start(
                tc1_in[:].rearrange("(p w) -> p w", p=128), comp1[:])
            nc.gpsimd.collective_compute(
                "AllGather", ALU.bypass,
                ins=[tc1_in[:]], outs=[tc1_full[:]],
                replica_groups=replica_groups,
            )
            _expand_table(nc, tc, cfg, tc1_full, tbl1, c.ROW1C, c.PITCH1)

            # ---------------- L1 edge phase + fused epilogue -----------
            gp_cm = tc.tile_pool(name="gat", bufs=5)
            gp = gp_cm.__enter__()
            tp_cm = tc.tile_pool(name="tmp", bufs=3)
            tpool = tp_cm.__enter__()
            psp_cm = tc.tile_pool(name="mmpb", bufs=2, space="PSUM")
            psp = psp_cm.__enter__()
            w2sb = pp.tile([32, 12], F32)
            nc.sync.dma_start(w2sb[:], w2e[:])

            LAG = 6
            idx_off = 0
            for b in range(NB):
                idx_off = _edge_block(nc, cfg, plan, gp, gip, wp, tpool,
                                      idxt, tbl1, a_d1, outcat, comp1, b,
                                      idx_off, layer=1)
                if b >= LAG:
                    _epilogue_block(nc, cfg, wp, psp, outcat, comp2, a_d2,
                                    b1t, w2sb, ident, pnt, pot, b - LAG)
            for b in range(NB - LAG, NB):
                _epilogue_block(nc, cfg, wp, psp, outcat, comp2, a_d2,
                                b1t, w2sb, ident, pnt, pot, b)
            psp_cm.__exit__(None, None, None)

            nc.sync.dma_start(
                tc2_in[:].rearrange("(p w) -> p w", p=128), comp2[:])
            nc.gpsimd.collective_compute(
                "AllGather", ALU.bypass,
                ins=[tc2_in[:]], outs=[tc2_full[:]],
                replica_groups=replica_groups,
            )
            _expand_table(nc, tc, cfg, tc2_full, tbl2, c.ROW2C, c.PITCH2)

            # ---------------- L2 edge phase + output -------------------
            def _final_block(b):
                S2 = out2cat[:, b * 9:b * 9 + 1]
                ou2 = out2cat[:, b * 9 + 1:(b + 1) * 9]
                r2 = wp.tile([128, 1], F32, tag="r2")
                nc.vector.reciprocal(r2[:], S2)
                fo = wp.tile([128, 8], F32, tag="fo")
                nc.vector.tensor_scalar(out=fo[:], in0=ou2, scalar1=r2[:],
                                        scalar2=None, op0=ALU.mult)
                nc.vector.tensor_add(fo[:], fo[:], b2t[:])
                nc.sync.dma_start(y[:, b * 8:(b + 1) * 8], fo[:])

            idx_off = 0
            for b in range(NB):
                idx_off = _edge_block(nc, cfg, plan, gp, gip, wp, tpool,
                                      idxt, tbl2, a_d2, out2cat, comp2, b,
                                      idx_off, layer=2)
                if b >= LAG:
                    _final_block(b - LAG)
            for b in range(NB - LAG, NB):
                _final_block(b)

            tp_cm.__exit__(None, None, None)
            gp_cm.__exit__(None, None, None)

    nc.finalize()
    return nc


def _dma_gather_raw(gps, out_ap, in_ap, idxs_ap, num_idxs, elem_size,
                    elem_step, queue_num):
    """bass.BassGpSimd.dma_gather with elem_size%256B relaxed to %4B."""
    from concourse import ap_utils
    from concourse.bass import MemorySpace
    import concourse.mybir as mb

    assert idxs_ap.dtype == I16
    assert in_ap.dtype == out_ap.dtype
    elem_size_bytes = elem_size * mb.dt.size(in_ap.dtype)
    assert elem_size_bytes > 0 and elem_size_bytes % 4 == 0
    assert in_ap.space == MemorySpace.DRAM
    assert idxs_ap.space == MemorySpace.SBUF
    assert out_ap.space == MemorySpace.SBUF
    assert ap_utils.ap_is_contiguous(out_ap.ap[1:])
    assert ap_utils.ap_is_contiguous(idxs_ap.ap[1:])
    assert in_ap.ap[-1][1] == out_ap.ap[-1][1] == elem_size
    assert out_ap.ap[0][1] * out_ap.ap[1][1] == ((num_idxs + 127) // 128) * 128
    assert in_ap.ap[0][0] == elem_step
    stride_bytes = elem_step * mb.dt.size(in_ap.dtype)
    assert stride_bytes % 256 == 0
    stride_bytes_256 = stride_bytes // 256
    assert stride_bytes_256 < 256

    _in_ap = gps.lower_ap_dma(in_ap, for_custom_bir_dma=True)
    _idxs_ap = gps.lower_ap(idxs_ap)
    _out_ap = gps.lower_ap(out_ap)
    return gps.add_instruction(
        mb.InstDMAGatherAnt(
            name=gps.bass.get_next_instruction_name(),
            ins=[*_in_ap, _idxs_ap,
                 gps.lower_val_access(gps.to_reg(num_idxs))],
            outs=[_out_ap],
            transpose=False,
            num_idxs=num_idxs,
            elem_size=elem_size,
            stride_bytes_256=stride_bytes_256,
            gen_mode=0,
            single_packet=False,
            queue_num=queue_num,
        )
    )


def _expand_table(nc, tc, cfg, compact_dram, padded_dram, roww, pitch):
    """Expand compact bf16 rows [TROWS, roww] (flat) to [TROWS, pitch]."""
    c = cfg
    R128 = c.TROWS // 128
    CH = 8
    while R128 % CH != 0:
        CH //= 2
    rch = R128 // CH
    ep_cm = tc.tile_pool(name=f"exp{roww}", bufs=2)
    ep = ep_cm.__enter__()
    for ch in range(CH):
        src = compact_dram[:].rearrange("(p r w) -> p r w", p=128, w=roww)
        ct = ep.tile([128, rch, roww], BF16, tag="exp_in")
        nc.sync.dma_start(ct[:], src[:, ch * rch:(ch + 1) * rch, :])
        pt = ep.tile([128, rch, pitch], BF16, tag="exp_out")
        nc.vector.memset(pt[:, :, roww:], 0.0)
        nc.vector.tensor_copy(pt[:, :, :roww], ct[:])
        dstv = padded_dram[:].rearrange("(p r) w -> p r w", p=128)
        nc.sync.dma_start(dstv[:, ch * rch:(ch + 1) * rch, :], pt[:])
    ep_cm.__exit__(None, None, None)


def _edge_block(nc, cfg, plan, gp, gip, wp, tpool, idxt, tbl, a_d, outcat,
                comp, b, idx_off, layer):
    """One block's gathers + attention + aggregation."""
    c = cfg
    H = c.H1 if layer == 1 else c.H2
    CC = c.C1 if layer == 1 else c.C2
    aw = 4 if layer == 1 else 1
    xw = H * CC
    RW = c.RW1 if layer == 1 else c.RW2
    pitch = c.PITCH1 if layer == 1 else c.PITCH2
    Dmax = plan.Dmax
    Dt = int(plan.Dg[b].sum())

    Gf = gp.tile([128, Dmax, RW], BF16, tag=f"G{layer}")
    G = Gf[:, :Dt, :]
    itf = gip.tile([128, 8 * Dmax], I16, tag="it")
    itb = itf[:, :8 * Dt]
    nc.sync.dma_start(itb[:], idxt[:, idx_off:idx_off + 8 * Dt])
    idx_off += 8 * Dt
    # class-q gather: rows r===q (mod 4) at stride 4*pitch
    tblv = tbl[:].rearrange("(k f) w -> k (f w)", f=4)  # [QROWS, 4*pitch]
    off = 0
    for q in range(4):
        Dq = int(plan.Dg[b, q])
        nsl = 128 * Dq
        _dma_gather_raw(
            nc.gpsimd,
            G[:, off:off + Dq, :],
            tblv[:, q * pitch:q * pitch + RW],
            itb[:, 8 * off:8 * (off + Dq)], nsl, RW, 4 * pitch,
            queue_num=q,
        )
        off += Dq

    # attention: lrelu on DVE (ACT Lrelu ignores alpha on HW), Exp+accum on
    # ScalarE (keeping ACT single-function avoids act-table reloads)
    Hm = cfg.H1
    pvf = tpool.tile([128, Hm, Dmax], BF16, tag="pv")
    pv = pvf[:, :H, :Dt]
    for h in range(H):
        adh = a_d[:, b * H + h:b * H + h + 1]
        ef2 = wp.tile([128, Dmax], F32, tag="e")
        ev = ef2[:, :Dt]
        nc.vector.tensor_scalar(out=ev, in0=G[:, :, h], scalar1=adh,
                                scalar2=None, op0=ALU.add)
        mf = wp.tile([128, Dmax], F32, tag="m")
        mv = mf[:, :Dt]
        nc.vector.tensor_scalar(out=mv, in0=ev, scalar1=0.0,
                                scalar2=c.NEG_SLOPE, op0=ALU.min,
                                op1=ALU.mult)
        lrf = wp.tile([128, Dmax], F32, tag="lr")
        lr = lrf[:, :Dt]
        nc.vector.scalar_tensor_tensor(out=lr, in0=ev, scalar=0.0, in1=mv,
                                       op0=ALU.max, op1=ALU.add)
        nc.scalar.activation(
            pv[:, h, :], lr, ACTF.Exp,
            accum_out=outcat[:, b * (H + xw) + h:b * (H + xw) + h + 1])

    # msg = p * xp ; out_un = sum_j msg  (DVE)
    tmpf = tpool.tile([128, c.H1 * c.C1, Dmax], BF16, tag="tmp")
    tmp = tmpf[:, :H * CC, :Dt]
    nc.vector.tensor_tensor(
        out=tmp[:].rearrange("p (h cc) d -> p h cc d", h=H),
        in0=pv[:, :, None, :].broadcast_to([128, H, CC, Dt]),
        in1=G[:, :, aw:aw + xw].rearrange("p d (h cc) -> p h cc d", h=H),
        op=ALU.mult)
    Scol = outcat[:, b * (H + xw):b * (H + xw) + H]
    Ocol = outcat[:, b * (H + xw) + H:(b + 1) * (H + xw)]
    nc.vector.tensor_reduce(out=Ocol, in_=tmp[:], op=ALU.add, axis=AX)

    # self-loop handled locally: p_self = exp(lrelu(a_s_own + a_d))
    ROWC = c.ROW1C if layer == 1 else c.ROW2C
    a_s_own = comp[:, b * ROWC:b * ROWC + (4 if layer == 1 else 1)]
    a_s_own = a_s_own[:, :H] if layer == 1 else a_s_own
    xp_own = comp[:, b * ROWC + aw:b * ROWC + aw + xw]
    esf = wp.tile([128, c.H1], F32, tag="es")
    es = esf[:, :H]
    nc.vector.tensor_add(es, a_s_own, a_d[:, b * H:(b + 1) * H])
    lrsf = wp.tile([128, c.H1], F32, tag="lrs")
    lrs = lrsf[:, :H]
    nc.vector.tensor_scalar(out=lrs, in0=es, scalar1=0.0,
                            scalar2=c.NEG_SLOPE, op0=ALU.min, op1=ALU.mult)
    nc.vector.tensor_scalar(out=es, in0=es, scalar1=0.0, scalar2=None,
                            op0=ALU.max)
    nc.vector.tensor_add(es, es, lrs)
    pslf = wp.tile([128, c.H1], F32, tag="psl")
    psl = pslf[:, :H]
    nc.scalar.activation(psl, es, ACTF.Exp)
    nc.vector.tensor_add(Scol, Scol, psl)
    mslf = wp.tile([128, c.H1 * c.C1], F32, tag="msl")
    msl = mslf[:, :H * CC]
    nc.vector.tensor_tensor(
        out=msl.rearrange("p (h cc) -> p h cc", h=H),
        in0=psl[:, :, None].broadcast_to([128, H, CC]),
        in1=xp_own.rearrange("p (h cc) -> p h cc", h=H),
        op=ALU.mult)
    nc.vector.tensor_add(Ocol, Ocol, msl)
    return idx_off


def _epilogue_block(nc, cfg, wp, psp, outcat, comp2, a_d2, b1t, w2sb, ident,
                    pnt, pot, b):
    """h = elu(out1/S + b1); xps2 = h @ W2ext; fill comp2 + a_d2."""
    c = cfg
    NB = c.NBLK
    S = outcat[:, b * 36:b * 36 + 4]
    if b >= NB - 2:
        nc.vector.tensor_add(S, S, pot[:].broadcast_to([128, 4]))
    ou = outcat[:, b * 36 + 4:(b + 1) * 36]
    r = wp.tile([128, 4], F32, tag="r1")
    nc.vector.reciprocal(r[:], S)
    z = wp.tile([128, 32], F32, tag="z")
    nc.vector.tensor_tensor(
        out=z[:].rearrange("p (h cc) -> p h cc", h=4),
        in0=ou.rearrange("p (h cc) -> p h cc", h=4),
        in1=r[:, :, None].broadcast_to([128, 4, 8]),
        op=ALU.mult)
    nc.vector.tensor_add(z[:], z[:], b1t[:])
    # elu: h = max(z,0) + exp(min(z,0)) - 1
    mneg = wp.tile([128, 32], F32, tag="mneg")
    nc.vector.tensor_scalar(out=mneg[:], in0=z[:], scalar1=0.0,
                            scalar2=None, op0=ALU.min)
    qe = wp.tile([128, 32], F32, tag="qe")
    nc.scalar.activation(qe[:], mneg[:], ACTF.Exp)
    hh = wp.tile([128, 32], F32, tag="hh")
    nc.vector.tensor_scalar(out=hh[:], in0=z[:], scalar1=0.0,
                            scalar2=None, op0=ALU.max)
    nc.vector.tensor_add(hh[:], hh[:], qe[:])
    nc.vector.tensor_scalar_add(hh[:], hh[:], -1.0)
    # xps2 = h @ W2ext
    psh = psp.tile([32, 128], F32, tag="trh")
    nc.tensor.transpose(psh[:], hh[:], ident[:])
    hT = wp.tile([32, 128], F32, tag="hT")
    nc.scalar.copy(hT[:], psh[:])
    ps2 = psp.tile([12, 128], F32, tag="mm2")
    nc.tensor.matmul(ps2[:], w2sb[:], hT[:], start=True, stop=True)
    x2T = wp.tile([12, 128], F32, tag="x2T")
    nc.scalar.copy(x2T[:], ps2[:])
    ps3 = psp.tile([128, 12], F32, tag="tr2")
    nc.tensor.transpose(ps3[:], x2T[:], ident[:12, :12])
    nm2 = wp.tile([128, 12], F32, tag="nm2")
    nc.scalar.copy(nm2[:], ps3[:])
    if b >= NB - 2:
        nc.vector.tensor_add(nm2[:, 8:9], nm2[:, 8:9], pnt[:])
    nc.vector.tensor_copy(a_d2[:, b:b + 1], nm2[:, 9:10])
    nc.vector.tensor_copy(comp2[:, b * 10:b * 10 + 1], nm2[:, 8:9])
    nc.vector.tensor_copy(comp2[:, b * 10 + 1:b * 10 + 9], nm2[:, 0:8])
    nc.vector.memset(comp2[:, b * 10 + 9:b * 10 + 10], 0.0)


# ----------------------------------------------------------------------------
# Host wrapper
# ----------------------------------------------------------------------------
def _build_w1ext(W1, att_src1, att_dst1):
    H, C = att_src1.shape
    As = np.zeros((H * C, H), np.float32)
    Ad = np.zeros((H * C, H), np.float32)
    for h in range(H):
        As[h * C:(h + 1) * C, h] = att_src1[h]
        Ad[h * C:(h + 1) * C, h] = att_dst1[h]
    return np.concatenate([W1, W1 @ As, W1 @ Ad], axis=1).astype(np.float32)


def _build_w2ext(W2, att_src2, att_dst2):
    H, C = att_src2.shape
    As = att_src2.reshape(C, 1).astype(np.float32)
    Ad = att_dst2.reshape(C, 1).astype(np.float32)
    out = np.concatenate([W2, W2 @ As, W2 @ Ad,
                          np.zeros((32, 2), np.float32)], axis=1)
    return out.astype(np.float32)


def _padvec(val):
    v = np.zeros((128, 1), np.float32)
    v[PAD_P0:] = val
    return v


LAST_EXEC_NS = None


def kernel(x, edge_index, W1, att_src1, att_dst1, b1, W2, att_src2, att_dst2,
           b2):
    import ml_dtypes
    cfg = Cfg(N=x.shape[0], E=edge_index.shape[1], F_IN=x.shape[1])
    plan = Plan(cfg, np.asarray(edge_index))
    nc = build_kernel(cfg, plan)

    x = np.asarray(x, dtype=np.float32)
    w1e = _build_w1ext(np.asarray(W1), np.asarray(att_src1),
                       np.asarray(att_dst1))
    w2e = _build_w2ext(np.asarray(W2), np.asarray(att_src2),
                       np.asarray(att_dst2))
    b1r = np.tile(np.asarray(b1, np.float32)[None, :], (128, 1))
    b2r = np.tile(np.asarray(b2, np.float32)[None, :], (128, 1))

    in_maps = []
    for ci in range(cfg.NCORES):
        posn = plan.pos_node[ci]                  # [NBLK, 128]
        xs = np.zeros((cfg.NPAD, cfg.F_IN), np.float32)
        flat = posn.reshape(-1)                   # rank = 128*b + p order
        real = flat >= 0
        xs[real] = x[flat[real]]
        in_maps.append({
            "xT": np.ascontiguousarray(xs.T).astype(ml_dtypes.bfloat16),
            "w1e": w1e.astype(ml_dtypes.bfloat16), "w2e": w2e,
            "b1r": b1r, "b2r": b2r,
            "padneg": _padvec(NEG_BIG), "padone": _padvec(1.0),
            "idxt": plan.idx_planes[ci],
        })

    global LAST_EXEC_NS
    want_trace = False
    try:
        from antenv.axon_hooks import get_axon_ntff_profile_hook
        want_trace = get_axon_ntff_profile_hook() is not None
    except ImportError:
        pass
    res = run_bass_kernel_spmd(nc, in_maps, core_ids=list(range(cfg.NCORES)),
                               trace=want_trace)
    LAST_EXEC_NS = res.exec_time_ns

    out = np.empty((cfg.N, 8), np.float32)
    for ci in range(cfg.NCORES):
        yv = res.results[ci]["y"].reshape(128, cfg.NBLK, 8)
        posn = plan.pos_node[ci]
        for b in range(cfg.NBLK):
            nodes = posn[b]
            m = nodes >= 0
            out[nodes[m]] = yv[np.nonzero(m)[0], b, :]
    return out


if __name__ == "__main__":
    pass


# revision 10
# speedup vs baseline: 2.0181x; 1.0457x over previous
"""2-layer GAT (PyG-style, eval mode) on 8 Trainium2 NeuronCores via Bass/Tile.

v2 architecture (dst-sharded, class-balanced gathers, bf16 tables):
  - Destination nodes sharded across 8 cores (12500 each, padded 12544).
  - Per core, dst nodes are sorted by in-degree and placed into 98 blocks of
    128 (partition dim). Table row of the node at (block b, partition p) is
    rc = 98*p + b (+ core*12544), so its "class" rc%4 == (2p+b)%4 is chosen
    by placement inside a 4-block superblock.
  - The gather for class q addresses rows r===q (mod 4) via base offset
    q*PITCH and stride 4*PITCH (int16 idx = r//4 <= 25087). A greedy
    host-side coloring balances each dst node's in-edge classes, cutting the
    per-(block,class) max-degree slot padding from ~2.06x to ~1.2x.
  - Tables are bf16: L1 rows 64 words (128B pitch): [a_s1(4) | xp1(32)];
    L2 rows 32 words (64B pitch): [a_s2(1) | xp2(8)].
  - Attention on ScalarE: p = Exp(Lrelu(a_s + a_d)) with fused bias + alpha
    and fused accumulation S = sum_j p. DVE only does msg = p*xp and the
    [128, H*C, D] segment reduce.
  - Padded slots point at a per-class "dummy" pad row with a_s = -1e30.
"""

import sys

sys.path.insert(0, "/opt/trn_rl_repo")

import numpy as np

import concourse.bass as bass
import concourse.bacc as bacc
import concourse.mybir as mybir
from concourse.tile import TileContext
from concourse import library_config
from concourse.bass_utils import run_bass_kernel_spmd

F32 = mybir.dt.float32
BF16 = mybir.dt.bfloat16
I16 = mybir.dt.int16

AX = mybir.AxisListType.X
ALU = mybir.AluOpType
ACTF = mybir.ActivationFunctionType

NEG_BIG = -1.0e30
PAD_P0 = 106          # pad slots: blocks NBLK-2, NBLK-1, partitions >= 106


class Cfg:
    def __init__(self, N=100000, E=3200000, F_IN=512, ncores=8):
        self.N = N
        self.E = E
        self.F_IN = F_IN
        self.H1, self.C1 = 4, 8
        self.H2, self.C2 = 1, 8
        self.NEG_SLOPE = 0.2
        self.NCORES = ncores
        assert N % ncores == 0
        self.NPC = N // ncores                       # real nodes per core
        self.NPAD = ((self.NPC + 127) // 128) * 128  # padded (12544)
        self.NBLK = self.NPAD // 128                 # 98
        self.TROWS = self.NPAD * ncores              # 100352
        self.QROWS = self.TROWS // 4                 # 25088 (per class)
        assert self.QROWS <= 32767
        self.PITCH1 = 64    # bf16 words per L1 table row (128B)
        self.PITCH2 = 32    # bf16 words per L2 table row (64B)
        self.RW1 = 36       # gathered words L1: [a_s1(4) | xp1(32)]
        self.RW2 = 10       # gathered words L2: [a_s2(1) | xp2(8) | pad]
        self.ROW1C = 36     # compact words L1
        self.ROW2C = 10     # compact words L2


# ----------------------------------------------------------------------------
# Host-side preprocessing
# ----------------------------------------------------------------------------
class Plan:
    """Sharding, class coloring, placement, slot/idx construction."""

    def __init__(self, cfg: Cfg, edge_index: np.ndarray):
        c = cfg
        # self-loops are handled locally on-device; edges only here
        src = edge_index[0].astype(np.int64)
        dst = edge_index[1].astype(np.int64)
        deg = np.bincount(dst, minlength=c.N)
        NSB = (c.NBLK + 3) // 4                     # superblocks per core

        # ---- superblock of each node (degree-descending order) ----------
        sb_of = np.zeros(c.N, dtype=np.int32)
        core_order = []                             # deg-sorted nodes per core
        for ci in range(c.NCORES):
            nodes = np.arange(ci * c.NPC, (ci + 1) * c.NPC)
            ns = nodes[np.argsort(-deg[nodes], kind="stable")]
            core_order.append(ns)
            sb_of[ns] = ci * NSB + np.minimum(np.arange(c.NPC) // 512,
                                              NSB - 1)

        # per-(core,sb,class) real capacity
        cap = np.full((c.NCORES * NSB, 4), 128, dtype=np.int64)
        # last sb: 2 blocks, 53 real slots per class
        last_real = c.NPC - (NSB - 1) * 512
        cap[NSB - 1::NSB, :] = last_real // 4
        assert last_real % 4 == 0

        # ---- class coloring (greedy + 1 refine pass) --------------------
        o = np.argsort(src, kind="stable")
        dst_sorted = dst[o].astype(np.int32)
        starts = np.searchsorted(src[o], np.arange(c.N + 1))
        cnt = np.zeros((c.N, 4), dtype=np.int16)
        cls = np.full(c.N, -1, dtype=np.int8)
        outdeg = np.bincount(src, minlength=c.N)
        order = np.argsort(-outdeg, kind="stable")
        tgt = np.ceil(deg / 4).astype(np.int16)
        for it in range(2):
            for s in order:
                ds = dst_sorted[starts[s]:starts[s + 1]]
                sb = sb_of[s]
                if it > 0:
                    q0 = cls[s]
                    cnt[ds, q0] -= 1
                    cap[sb, q0] += 1
                cc = cnt[ds, :].astype(np.int32)
                over = (cc + 1 > tgt[ds][:, None]).sum(axis=0) * 1000 \
                    + cc.sum(axis=0)
                for q in np.argsort(over, kind="stable"):
                    if cap[sb, q] > 0:
                        cls[s] = q
                        cap[sb, q] -= 1
                        cnt[ds, q] += 1
                        break
        self.cls = cls

        # ---- placement: node -> (block, partition) ----------------------
        # Within block b: even p -> class b%4, odd p -> class (b+2)%4.
        # Pad slots (no node): blocks NBLK-2/NBLK-1, p >= PAD_P0.
        self.pos_node = []
        b_of = np.empty(c.N, dtype=np.int64)
        p_of = np.empty(c.N, dtype=np.int64)
        for ci in range(c.NCORES):
            ns = core_order[ci]
            posn = np.full((c.NBLK, 128), -1, dtype=np.int64)
            for sbi in range(NSB):
                blocks = range(4 * sbi, min(4 * sbi + 4, c.NBLK))
                lo = sbi * 512
                hi = min(lo + 512, c.NPC)
                sb_nodes = ns[lo:hi]
                slots = {q: [] for q in range(4)}
                for b in blocks:
                    for p in range(128):
                        if b >= c.NBLK - 2 and p >= PAD_P0:
                            continue
                        slots[(2 * p + b) % 4].append((b, p))
                byq = {q: [] for q in range(4)}
                for n in sb_nodes:
                    byq[int(cls[n])].append(n)
                for q in range(4):
                    qn = sorted(byq[q], key=lambda n: -int(cnt[n].max()))
                    assert len(qn) <= len(slots[q])
                    for n, (b, p) in zip(qn, slots[q]):
                        posn[b, p] = n
                        b_of[n] = b
                        p_of[n] = p
            self.pos_node.append(posn)

        # ---- table rows & per-class dummy (pad) rows --------------------
        core_of_node = np.arange(c.N) // c.NPC
        self.row_of = core_of_node * c.NPAD + c.NBLK * p_of + b_of
        self.dummy_idx = np.full(4, -1, dtype=np.int64)
        for b in (c.NBLK - 2, c.NBLK - 1):
            for p in range(PAD_P0, 128):
                q = (2 * p + b) % 4
                if self.dummy_idx[q] < 0:
                    self.dummy_idx[q] = (c.NBLK * p + b) // 4
        assert (self.dummy_idx >= 0).all()

        # ---- per-core counts -> global Dg -------------------------------
        cls_src = cls[src].astype(np.int64)
        idx_src = (self.row_of[src] // 4).astype(np.int64)
        per_core = []
        D_all = np.zeros((c.NCORES, c.NBLK, 4), dtype=np.int64)
        core_of_dst = dst // c.NPC
        for ci in range(c.NCORES):
            m = core_of_dst == ci
            bb = b_of[dst[m]]
            pp = p_of[dst[m]]
            qq = cls_src[m]
            ii = idx_src[m]
            counts = np.zeros((c.NBLK, 4, 128), dtype=np.int64)
            np.add.at(counts, (bb, qq, pp), 1)
            D_all[ci] = counts.max(axis=2)
            per_core.append((bb, pp, qq, ii))
        self.Dg = np.maximum(D_all.max(axis=0), 1)   # [NBLK, 4]
        self.Dmax = int(self.Dg.sum(1).max())
        self.tot_slots = int(128 * self.Dg.sum())

        # ---- idx planes -------------------------------------------------
        self.idx_planes = []
        for ci in range(c.NCORES):
            bb, pp, qq, ii = per_core[ci]
            key = (bb * 4 + qq) * 128 + pp
            ordk = np.argsort(key, kind="stable")
            key_s = key[ordk]
            ii_s = ii[ordk]
            pp_s = pp[ordk]
            bq = key_s // 128
            seg_lo = np.searchsorted(bq, np.arange(c.NBLK * 4))
            seg_hi = np.searchsorted(bq, np.arange(c.NBLK * 4) + 1)
            segs = []
            for b in range(c.NBLK):
                for q in range(4):
                    k = b * 4 + q
                    nslots = 128 * int(self.Dg[b, q])
                    arr = np.full(nslots, self.dummy_idx[q], dtype=np.int16)
                    pseg = pp_s[seg_lo[k]:seg_hi[k]]
                    iseg = ii_s[seg_lo[k]:seg_hi[k]]
                    jj = np.arange(len(pseg)) - np.searchsorted(pseg, pseg)
                    arr[jj * 128 + pseg] = iseg.astype(np.int16)
                    segs.append(arr.reshape(-1, 16).T)
            wrapped = np.concatenate(segs, axis=1)
            self.idx_planes.append(np.tile(wrapped, (8, 1)).astype(np.int16))
        self.idx_cols = self.idx_planes[0].shape[1]


# ----------------------------------------------------------------------------
# Device kernel builder (one program, SPMD on 8 cores)
# ----------------------------------------------------------------------------
def build_kernel(cfg: Cfg, plan: Plan):
    c = cfg
    NB = c.NBLK
    nc = bacc.Bacc(num_swdge_queues=4, num_devices=c.NCORES)

    xT = nc.dram_tensor("xT", [c.F_IN, c.NPAD], BF16, kind="ExternalInput")
    w1e = nc.dram_tensor("w1e", [c.F_IN, 40], BF16, kind="ExternalInput")
    w2e = nc.dram_tensor("w2e", [32, 12], F32, kind="ExternalInput")
    b1r = nc.dram_tensor("b1r", [128, 32], F32, kind="ExternalInput")
    b2r = nc.dram_tensor("b2r", [128, 8], F32, kind="ExternalInput")
    padneg = nc.dram_tensor("padneg", [128, 1], F32, kind="ExternalInput")
    padone = nc.dram_tensor("padone", [128, 1], F32, kind="ExternalInput")
    idxt = nc.dram_tensor("idxt", [128, plan.idx_cols], I16,
                          kind="ExternalInput")
    y = nc.dram_tensor("y", [128, NB * 8], F32, kind="ExternalOutput")

    tc1_in = nc.dram_tensor("tc1_in", [128 * NB * c.ROW1C], BF16,
                            kind="Internal")
    tc1_full = nc.dram_tensor("tc1_full", [c.TROWS * c.ROW1C], BF16,
                              kind="Internal", addr_space="Shared")
    tbl1 = nc.dram_tensor("tbl1", [c.TROWS, c.PITCH1], BF16, kind="Internal")
    tc2_in = nc.dram_tensor("tc2_in", [128 * NB * c.ROW2C], BF16,
                            kind="Internal")
    tc2_full = nc.dram_tensor("tc2_full", [c.TROWS * c.ROW2C], BF16,
                              kind="Internal", addr_space="Shared")
    tbl2 = nc.dram_tensor("tbl2", [c.TROWS, c.PITCH2], BF16, kind="Internal")

    replica_groups = [list(range(c.NCORES))]

    with TileContext(nc) as tc:
        with (
            tc.tile_pool(name="persist", bufs=1) as pp,
            tc.tile_pool(name="gidx", bufs=10) as gip,
            tc.tile_pool(name="work", bufs=3) as wp,
        ):
            with tc.high_priority():
                nc.gpsimd.load_library(library_config.mlp)

            a_d1 = pp.tile([128, NB * 4], F32)
            a_d2 = pp.tile([128, NB], F32)
            comp1 = pp.tile([128, NB * c.ROW1C], BF16)
            outcat = pp.tile([128, NB * 36], F32)   # [S(4) | out_un(32)]
            comp2 = pp.tile([128, NB * c.ROW2C], BF16)
            out2cat = pp.tile([128, NB * 9], F32)   # [S2(1) | out2_un(8)]
            b1t = pp.tile([128, 32], F32)
            b2t = pp.tile([128, 8], F32)
            pnt = pp.tile([128, 1], F32)
            pot = pp.tile([128, 1], F32)
            nc.sync.dma_start(b1t[:], b1r[:])
            nc.sync.dma_start(b2t[:], b2r[:])
            nc.sync.dma_start(pnt[:], padneg[:])
            nc.sync.dma_start(pot[:], padone[:])

            # ---------------- Phase A: xps1 = x @ W1ext ----------------
            w1sb = pp.tile([128, 4, 40], BF16)
            nc.sync.dma_start(w1sb[:],
                              w1e[:].rearrange("(k p) n -> p k n", p=128))
            ident = pp.tile([128, 128], F32)
            from concourse.masks import make_identity
            make_identity(nc, ident[:])

            NT = 512
            mp_cm = tc.tile_pool(name="mm", bufs=3)
            mp = mp_cm.__enter__()
            psp_cm = tc.tile_pool(name="mmpa", bufs=2, space="PSUM")
            psp = psp_cm.__enter__()
            for t0 in range(0, c.NPAD, NT):
                nt = min(NT, c.NPAD - t0)
                xtile = mp.tile([128, 4, NT], BF16, tag="xt")
                nc.sync.dma_start(
                    xtile[:, :, :nt],
                    xT[:, t0:t0 + nt].rearrange("(k p) n -> p k n", p=128))
                ps = psp.tile([40, NT], F32, tag="mm1")
                for k in range(4):
                    nc.tensor.matmul(ps[:, :nt], w1sb[:, k, :],
                                     xtile[:, k, :nt],
                                     start=(k == 0), stop=(k == 3))
                xpsT = mp.tile([40, NT], F32, tag="xpsT")
                nc.scalar.copy(xpsT[:, :nt], ps[:, :nt])
                for s0 in range(0, nt, 128):
                    b = (t0 + s0) // 128
                    pst = psp.tile([128, 40], F32, tag="tr1")
                    nc.tensor.transpose(pst[:], xpsT[:, s0:s0 + 128],
                                        ident[:40, :40])
                    nm = wp.tile([128, 40], F32, tag="nm")
                    nc.scalar.copy(nm[:], pst[:])
                    if b >= NB - 2:
                        nc.vector.tensor_add(
                            nm[:, 32:36], nm[:, 32:36],
                            pnt[:].broadcast_to([128, 4]))
                    nc.vector.tensor_copy(a_d1[:, b * 4:(b + 1) * 4],
                                          nm[:, 36:40])
                    nc.vector.tensor_copy(
                        comp1[:, b * c.ROW1C:b * c.ROW1C + 4], nm[:, 32:36])
                    nc.vector.tensor_copy(
                        comp1[:, b * c.ROW1C + 4:(b + 1) * c.ROW1C],
                        nm[:, 0:32])
            psp_cm.__exit__(None, None, None)
            mp_cm.__exit__(None, None, None)

            nc.sync.dma_start(
                tc1_in[:].rearrange("(p w) -> p w", p=128), comp1[:])
            nc.gpsimd.collective_compute(
                "AllGather", ALU.bypass,
                ins=[tc1_in[:]], outs=[tc1_full[:]],
                replica_groups=replica_groups,
            )
            _expand_table(nc, tc, cfg, tc1_full, tbl1, c.ROW1C, c.PITCH1)

            # ---------------- L1 edge phase + fused epilogue -----------
            gp_cm = tc.tile_pool(name="gat", bufs=7)
            gp = gp_cm.__enter__()
            tp_cm = tc.tile_pool(name="tmp", bufs=4)
            tpool = tp_cm.__enter__()
            psp_cm = tc.tile_pool(name="mmpb", bufs=2, space="PSUM")
            psp = psp_cm.__enter__()
            w2sb = pp.tile([32, 12], F32)
            nc.sync.dma_start(w2sb[:], w2e[:])

            LAG = 6
            idx_off = 0
            for b in range(NB):
                idx_off = _edge_block(nc, cfg, plan, gp, gip, wp, tpool,
                                      idxt, tbl1, a_d1, outcat, comp1, b,
                                      idx_off, layer=1)
                if b >= LAG:
                    _epilogue_block(nc, cfg, wp, psp, outcat, comp2, a_d2,
                                    b1t, w2sb, ident, pnt, pot, b - LAG)
            for b in range(NB - LAG, NB):
                _epilogue_block(nc, cfg, wp, psp, outcat, comp2, a_d2,
                                b1t, w2sb, ident, pnt, pot, b)
            psp_cm.__exit__(None, None, None)

            nc.sync.dma_start(
                tc2_in[:].rearrange("(p w) -> p w", p=128), comp2[:])
            nc.gpsimd.collective_compute(
                "AllGather", ALU.bypass,
                ins=[tc2_in[:]], outs=[tc2_full[:]],
                replica_groups=replica_groups,
            )
            _expand_table(nc, tc, cfg, tc2_full, tbl2, c.ROW2C, c.PITCH2)

            # ---------------- L2 edge phase + output -------------------
            def _final_block(b):
                S2 = out2cat[:, b * 9:b * 9 + 1]
                ou2 = out2cat[:, b * 9 + 1:(b + 1) * 9]
                r2 = wp.tile([128, 1], F32, tag="r2")
                nc.vector.reciprocal(r2[:], S2)
                fo = wp.tile([128, 8], F32, tag="fo")
                nc.vector.tensor_scalar(out=fo[:], in0=ou2, scalar1=r2[:],
                                        scalar2=None, op0=ALU.mult)
                nc.vector.tensor_add(fo[:], fo[:], b2t[:])
                nc.sync.dma_start(y[:, b * 8:(b + 1) * 8], fo[:])

            idx_off = 0
            for b in range(NB):
                idx_off = _edge_block(nc, cfg, plan, gp, gip, wp, tpool,
                                      idxt, tbl2, a_d2, out2cat, comp2, b,
                                      idx_off, layer=2)
                if b >= LAG:
                    _final_block(b - LAG)
            for b in range(NB - LAG, NB):
                _final_block(b)

            tp_cm.__exit__(None, None, None)
            gp_cm.__exit__(None, None, None)

    nc.finalize()
    return nc


def _dma_gather_raw(gps, out_ap, in_ap, idxs_ap, num_idxs, elem_size,
                    elem_step, queue_num):
    """bass.BassGpSimd.dma_gather with elem_size%256B relaxed to %4B."""
    from concourse import ap_utils
    from concourse.bass import MemorySpace
    import concourse.mybir as mb

    assert idxs_ap.dtype == I16
    assert in_ap.dtype == out_ap.dtype
    elem_size_bytes = elem_size * mb.dt.size(in_ap.dtype)
    assert elem_size_bytes > 0 and elem_size_bytes % 4 == 0
    assert in_ap.space == MemorySpace.DRAM
    assert idxs_ap.space == MemorySpace.SBUF
    assert out_ap.space == MemorySpace.SBUF
    assert ap_utils.ap_is_contiguous(out_ap.ap[1:])
    assert ap_utils.ap_is_contiguous(idxs_ap.ap[1:])
    assert in_ap.ap[-1][1] == out_ap.ap[-1][1] == elem_size
    assert out_ap.ap[0][1] * out_ap.ap[1][1] == ((num_idxs + 127) // 128) * 128
    assert in_ap.ap[0][0] == elem_step
    stride_bytes = elem_step * mb.dt.size(in_ap.dtype)
    assert stride_bytes % 256 == 0
    stride_bytes_256 = stride_bytes // 256
    assert stride_bytes_256 < 256

    _in_ap = gps.lower_ap_dma(in_ap, for_custom_bir_dma=True)
    _idxs_ap = gps.lower_ap(idxs_ap)
    _out_ap = gps.lower_ap(out_ap)
    return gps.add_instruction(
        mb.InstDMAGatherAnt(
            name=gps.bass.get_next_instruction_name(),
            ins=[*_in_ap, _idxs_ap,
                 gps.lower_val_access(gps.to_reg(num_idxs))],
            outs=[_out_ap],
            transpose=False,
            num_idxs=num_idxs,
            elem_size=elem_size,
            stride_bytes_256=stride_bytes_256,
            gen_mode=0,
            single_packet=False,
            queue_num=queue_num,
        )
    )


def _expand_table(nc, tc, cfg, compact_dram, padded_dram, roww, pitch):
    """Expand compact bf16 rows [TROWS, roww] (flat) to [TROWS, pitch]."""
    c = cfg
    R128 = c.TROWS // 128
    CH = 8
    while R128 % CH != 0:
        CH //= 2
    rch = R128 // CH
    ep_cm = tc.tile_pool(name=f"exp{roww}", bufs=2)
    ep = ep_cm.__enter__()
    for ch in range(CH):
        src = compact_dram[:].rearrange("(p r w) -> p r w", p=128, w=roww)
        ct = ep.tile([128, rch, roww], BF16, tag="exp_in")
        nc.sync.dma_start(ct[:], src[:, ch * rch:(ch + 1) * rch, :])
        pt = ep.tile([128, rch, pitch], BF16, tag="exp_out")
        nc.vector.memset(pt[:, :, roww:], 0.0)
        nc.vector.tensor_copy(pt[:, :, :roww], ct[:])
        dstv = padded_dram[:].rearrange("(p r) w -> p r w", p=128)
        nc.sync.dma_start(dstv[:, ch * rch:(ch + 1) * rch, :], pt[:])
    ep_cm.__exit__(None, None, None)


def _edge_block(nc, cfg, plan, gp, gip, wp, tpool, idxt, tbl, a_d, outcat,
                comp, b, idx_off, layer):
    """One block's gathers + attention + aggregation."""
    c = cfg
    H = c.H1 if layer == 1 else c.H2
    CC = c.C1 if layer == 1 else c.C2
    aw = 4 if layer == 1 else 1
    xw = H * CC
    RW = c.RW1 if layer == 1 else c.RW2
    pitch = c.PITCH1 if layer == 1 else c.PITCH2
    Dmax = plan.Dmax
    Dt = int(plan.Dg[b].sum())

    Gf = gp.tile([128, Dmax, RW], BF16, tag=f"G{layer}")
    G = Gf[:, :Dt, :]
    itf = gip.tile([128, 8 * Dmax], I16, tag="it")
    itb = itf[:, :8 * Dt]
    nc.sync.dma_start(itb[:], idxt[:, idx_off:idx_off + 8 * Dt])
    idx_off += 8 * Dt
    # class-q gather: rows r===q (mod 4) at stride 4*pitch
    tblv = tbl[:].rearrange("(k f) w -> k (f w)", f=4)  # [QROWS, 4*pitch]
    off = 0
    for q in range(4):
        Dq = int(plan.Dg[b, q])
        nsl = 128 * Dq
        _dma_gather_raw(
            nc.gpsimd,
            G[:, off:off + Dq, :],
            tblv[:, q * pitch:q * pitch + RW],
            itb[:, 8 * off:8 * (off + Dq)], nsl, RW, 4 * pitch,
            queue_num=q,
        )
        off += Dq

    # attention: lrelu on DVE (ACT Lrelu ignores alpha on HW), Exp+accum on
    # ScalarE (keeping ACT single-function avoids act-table reloads)
    Hm = cfg.H1
    pvf = tpool.tile([128, Hm, Dmax], BF16, tag="pv")
    pv = pvf[:, :H, :Dt]
    ef2 = wp.tile([128, Hm, Dmax], F32, tag="e")
    ev = ef2[:, :H, :Dt]
    nc.vector.tensor_tensor(
        out=ev, in0=G[:, :, :H].rearrange("p d h -> p h d"),
        in1=a_d[:, b * H:(b + 1) * H, None].broadcast_to([128, H, Dt]),
        op=ALU.add)
    mf = wp.tile([128, Hm, Dmax], F32, tag="m")
    mv = mf[:, :H, :Dt]
    nc.vector.tensor_scalar(out=mv, in0=ev, scalar1=0.0,
                            scalar2=c.NEG_SLOPE, op0=ALU.min, op1=ALU.mult)
    lrf = wp.tile([128, Hm, Dmax], F32, tag="lr")
    lr = lrf[:, :H, :Dt]
    nc.vector.scalar_tensor_tensor(out=lr, in0=ev, scalar=0.0, in1=mv,
                                   op0=ALU.max, op1=ALU.add)
    for h in range(H):
        nc.scalar.activation(
            pv[:, h, :], lr[:, h, :], ACTF.Exp,
            accum_out=outcat[:, b * (H + xw) + h:b * (H + xw) + h + 1])

    # msg = p * xp ; out_un = sum_j msg  (DVE)
    tmpf = tpool.tile([128, c.H1 * c.C1, Dmax], BF16, tag="tmp")
    tmp = tmpf[:, :H * CC, :Dt]
    nc.vector.tensor_tensor(
        out=tmp[:].rearrange("p (h cc) d -> p h cc d", h=H),
        in0=pv[:, :, None, :].broadcast_to([128, H, CC, Dt]),
        in1=G[:, :, aw:aw + xw].rearrange("p d (h cc) -> p h cc d", h=H),
        op=ALU.mult)
    Scol = outcat[:, b * (H + xw):b * (H + xw) + H]
    Ocol = outcat[:, b * (H + xw) + H:(b + 1) * (H + xw)]
    nc.vector.tensor_reduce(out=Ocol, in_=tmp[:], op=ALU.add, axis=AX)

    # self-loop handled locally: p_self = exp(lrelu(a_s_own + a_d))
    ROWC = c.ROW1C if layer == 1 else c.ROW2C
    a_s_own = comp[:, b * ROWC:b * ROWC + (4 if layer == 1 else 1)]
    a_s_own = a_s_own[:, :H] if layer == 1 else a_s_own
    xp_own = comp[:, b * ROWC + aw:b * ROWC + aw + xw]
    esf = wp.tile([128, c.H1], F32, tag="es")
    es = esf[:, :H]
    nc.vector.tensor_add(es, a_s_own, a_d[:, b * H:(b + 1) * H])
    lrsf = wp.tile([128, c.H1], F32, tag="lrs")
    lrs = lrsf[:, :H]
    nc.vector.tensor_scalar(out=lrs, in0=es, scalar1=0.0,
                            scalar2=c.NEG_SLOPE, op0=ALU.min, op1=ALU.mult)
    nc.vector.tensor_scalar(out=es, in0=es, scalar1=0.0, scalar2=None,
                            op0=ALU.max)
    nc.vector.tensor_add(es, es, lrs)
    pslf = wp.tile([128, c.H1], F32, tag="psl")
    psl = pslf[:, :H]
    nc.scalar.activation(psl, es, ACTF.Exp)
    nc.vector.tensor_add(Scol, Scol, psl)
    mslf = wp.tile([128, c.H1 * c.C1], F32, tag="msl")
    msl = mslf[:, :H * CC]
    nc.vector.tensor_tensor(
        out=msl.rearrange("p (h cc) -> p h cc", h=H),
        in0=psl[:, :, None].broadcast_to([128, H, CC]),
        in1=xp_own.rearrange("p (h cc) -> p h cc", h=H),
        op=ALU.mult)
    nc.vector.tensor_add(Ocol, Ocol, msl)
    return idx_off


def _epilogue_block(nc, cfg, wp, psp, outcat, comp2, a_d2, b1t, w2sb, ident,
                    pnt, pot, b):
    """h = elu(out1/S + b1); xps2 = h @ W2ext; fill comp2 + a_d2."""
    c = cfg
    NB = c.NBLK
    S = outcat[:, b * 36:b * 36 + 4]
    if b >= NB - 2:
        nc.vector.tensor_add(S, S, pot[:].broadcast_to([128, 4]))
    ou = outcat[:, b * 36 + 4:(b + 1) * 36]
    r = wp.tile([128, 4], F32, tag="r1")
    nc.vector.reciprocal(r[:], S)
    z = wp.tile([128, 32], F32, tag="z")
    nc.vector.tensor_tensor(
        out=z[:].rearrange("p (h cc) -> p h cc", h=4),
        in0=ou.rearrange("p (h cc) -> p h cc", h=4),
        in1=r[:, :, None].broadcast_to([128, 4, 8]),
        op=ALU.mult)
    nc.vector.tensor_add(z[:], z[:], b1t[:])
    # elu: h = max(z,0) + exp(min(z,0)) - 1
    mneg = wp.tile([128, 32], F32, tag="mneg")
    nc.vector.tensor_scalar(out=mneg[:], in0=z[:], scalar1=0.0,
                            scalar2=None, op0=ALU.min)
    qe = wp.tile([128, 32], F32, tag="qe")
    nc.scalar.activation(qe[:], mneg[:], ACTF.Exp)
    hh = wp.tile([128, 32], F32, tag="hh")
    nc.vector.tensor_scalar(out=hh[:], in0=z[:], scalar1=0.0,
                            scalar2=None, op0=ALU.max)
    nc.vector.tensor_add(hh[:], hh[:], qe[:])
    nc.vector.tensor_scalar_add(hh[:], hh[:], -1.0)
    # xps2 = h @ W2ext
    psh = psp.tile([32, 128], F32, tag="trh")
    nc.tensor.transpose(psh[:], hh[:], ident[:])
    hT = wp.tile([32, 128], F32, tag="hT")
    nc.scalar.copy(hT[:], psh[:])
    ps2 = psp.tile([12, 128], F32, tag="mm2")
    nc.tensor.matmul(ps2[:], w2sb[:], hT[:], start=True, stop=True)
    x2T = wp.tile([12, 128], F32, tag="x2T")
    nc.scalar.copy(x2T[:], ps2[:])
    ps3 = psp.tile([128, 12], F32, tag="tr2")
    nc.tensor.transpose(ps3[:], x2T[:], ident[:12, :12])
    nm2 = wp.tile([128, 12], F32, tag="nm2")
    nc.scalar.copy(nm2[:], ps3[:])
    if b >= NB - 2:
        nc.vector.tensor_add(nm2[:, 8:9], nm2[:, 8:9], pnt[:])
    nc.vector.tensor_copy(a_d2[:, b:b + 1], nm2[:, 9:10])
    nc.vector.tensor_copy(comp2[:, b * 10:b * 10 + 1], nm2[:, 8:9])
    nc.vector.tensor_copy(comp2[:, b * 10 + 1:b * 10 + 9], nm2[:, 0:8])
    nc.vector.memset(comp2[:, b * 10 + 9:b * 10 + 10], 0.0)


# ----------------------------------------------------------------------------
# Host wrapper
# ----------------------------------------------------------------------------
def _build_w1ext(W1, att_src1, att_dst1):
    H, C = att_src1.shape
    As = np.zeros((H * C, H), np.float32)
    Ad = np.zeros((H * C, H), np.float32)
    for h in range(H):
        As[h * C:(h + 1) * C, h] = att_src1[h]
        Ad[h * C:(h + 1) * C, h] = att_dst1[h]
    return np.concatenate([W1, W1 @ As, W1 @ Ad], axis=1).astype(np.float32)


def _build_w2ext(W2, att_src2, att_dst2):
    H, C = att_src2.shape
    As = att_src2.reshape(C, 1).astype(np.float32)
    Ad = att_dst2.reshape(C, 1).astype(np.float32)
    out = np.concatenate([W2, W2 @ As, W2 @ Ad,
                          np.zeros((32, 2), np.float32)], axis=1)
    return out.astype(np.float32)


def _padvec(val):
    v = np.zeros((128, 1), np.float32)
    v[PAD_P0:] = val
    return v


LAST_EXEC_NS = None


def kernel(x, edge_index, W1, att_src1, att_dst1, b1, W2, att_src2, att_dst2,
           b2):
    import ml_dtypes
    cfg = Cfg(N=x.shape[0], E=edge_index.shape[1], F_IN=x.shape[1])
    plan = Plan(cfg, np.asarray(edge_index))
    nc = build_kernel(cfg, plan)

    x = np.asarray(x, dtype=np.float32)
    w1e = _build_w1ext(np.asarray(W1), np.asarray(att_src1),
                       np.asarray(att_dst1))
    w2e = _build_w2ext(np.asarray(W2), np.asarray(att_src2),
                       np.asarray(att_dst2))
    b1r = np.tile(np.asarray(b1, np.float32)[None, :], (128, 1))
    b2r = np.tile(np.asarray(b2, np.float32)[None, :], (128, 1))

    in_maps = []
    for ci in range(cfg.NCORES):
        posn = plan.pos_node[ci]                  # [NBLK, 128]
        xs = np.zeros((cfg.NPAD, cfg.F_IN), np.float32)
        flat = posn.reshape(-1)                   # rank = 128*b + p order
        real = flat >= 0
        xs[real] = x[flat[real]]
        in_maps.append({
            "xT": np.ascontiguousarray(xs.T).astype(ml_dtypes.bfloat16),
            "w1e": w1e.astype(ml_dtypes.bfloat16), "w2e": w2e,
            "b1r": b1r, "b2r": b2r,
            "padneg": _padvec(NEG_BIG), "padone": _padvec(1.0),
            "idxt": plan.idx_planes[ci],
        })

    global LAST_EXEC_NS
    want_trace = False
    try:
        from antenv.axon_hooks import get_axon_ntff_profile_hook
        want_trace = get_axon_ntff_profile_hook() is not None
    except ImportError:
        pass
    res = run_bass_kernel_spmd(nc, in_maps, core_ids=list(range(cfg.NCORES)),
                               trace=want_trace)
    LAST_EXEC_NS = res.exec_time_ns

    out = np.empty((cfg.N, 8), np.float32)
    for ci in range(cfg.NCORES):
        yv = res.results[ci]["y"].reshape(128, cfg.NBLK, 8)
        posn = plan.pos_node[ci]
        for b in range(cfg.NBLK):
            nodes = posn[b]
            m = nodes >= 0
            out[nodes[m]] = yv[np.nonzero(m)[0], b, :]
    return out


if __name__ == "__main__":
    pass
